# revision 1
# baseline (speedup 1.0000x reference)
"""CCX loss kernel for Trainium2 (8 NeuronCores, data-parallel over batch).

Math (per batch element n, with C=256 channels, HW=64*64=4096 pixels):
  y_mu[c]   = mean over (n, h, w) of y            (host, tiny)
  x_c = x - y_mu ; y_c = y - y_mu                 (device)
  x_n = x_c/||x_c||_C ; y_n = y_c/||y_c||_C       (device)
  s[i,j]    = sum_c x_n[c,i] y_n[c,j]             (device matmul, f32r)
  d = 1-s ; dt = d/(dmin_i+eps) ; w = exp((1-dt)/0.5)
  ccx_ij = w/sum_j w ; ccx_n = mean_j max_i ccx_ij
  loss = mean_n -log(ccx_n + eps)                 (host, 8 scalars)

Key identities used on device:
  w_ij = exp(s*a_i + b_i),  a_i = 2/(dmin_i+eps), b_i = 2-a_i
  s*a_i = G*alpha_i with G = x_c^T y_n (unnormalized-x matmul),
          alpha_i = a_i/||x_c[:,i]||
  max_i ccx_ij = exp(max_i (G^T[j,i]*alpha_i + (b_i - lnZ_i)))

Schedule notes (perf):
  - PSUM holds one block row G[128, 4096] (8 banks).  Matmuls are
    ordered tile-major so each 2-bank tile completes early and its
    FD-512 reduce_max overlaps the remaining matmuls (short tail).
  - The softmax u = w/Z is shift-invariant, so the pass-1 exp runs with
    scale=alpha only (b1 cancels); pass-2's bias is just -lnZ'.  This
    shortens the per-block serial chain reduce->recip->alpha->exp that
    gates PSUM reuse (the exp is the last PSUM reader).
  - exp in place as 2x FD-2048 instructions; the next block's matmuls
    follow per freed 4-bank half, keeping PE idle gaps under the ~3.4us
    HAM re-throttle window (PE stays at 2.4 GHz).
"""

import os
import sys

import numpy as np

sys.path.insert(0, "/opt/trn_rl_repo")
os.environ.setdefault("JAX_PLATFORMS", "axon")

import concourse.mybir as mybir
import concourse.tile as tile
from concourse import bacc, bass_isa
from concourse.bass_utils import run_bass_kernel_spmd

N, C, H, W = 8, 256, 64, 64
HW = H * W          # 4096
NB = HW // 128      # 32 blocks of 128 rows/cols
EPS = 1e-6
F32 = mybir.dt.float32
F32R = mybir.dt.float32r
BF16 = mybir.dt.bfloat16
ALU = mybir.AluOpType
ACTF = mybir.ActivationFunctionType
AX = mybir.AxisListType
NEG_INF = -3.0e38

_cached = {}


def _build():
    nc = bacc.Bacc(None, target_bir_lowering=False, debug=True)
    xs = nc.dram_tensor("xs", [C, HW], F32, kind="ExternalInput")
    ys = nc.dram_tensor("ys", [C, HW], F32, kind="ExternalInput")
    ymu = nc.dram_tensor("ymu", [128, 2], F32, kind="ExternalInput")
    out = nc.dram_tensor("out", [1, 1], F32, kind="ExternalOutput")
    scr_y = nc.dram_tensor("scr_y", [NB, 128], F32)
    scr_a = nc.dram_tensor("scr_a", [NB, 128], F32)
    scr_b = nc.dram_tensor("scr_b", [NB, 128], F32)

    repeat = int(os.environ.get("BASS_REPEAT", "1"))
    with tile.TileContext(nc) as tc:
        import contextlib
        rep_ctx = tc.For_i(0, repeat, 1) if repeat > 1 else contextlib.nullcontext()
        with rep_ctx:
         with (
             tc.tile_pool(name="big", bufs=1) as big,
             tc.tile_pool(name="bc3", bufs=2) as bc3,
             tc.tile_pool(name="sq", bufs=2) as sqp,
             tc.tile_pool(name="small", bufs=1) as sm,
             tc.tile_pool(name="mmq", bufs=1, space="PSUM") as mmq,
         ):
             import concourse.bass as bass_mod

             # ---------------- load ----------------
             x = big.tile([128, 2, HW], F32, tag="x")
             xc = big.tile([128, 2, HW], F32R, tag="xc")
             y = big.tile([128, 2, HW], F32, tag="y")
             yn = big.tile([128, 2, HW], F32R, tag="yn")
             ymu_sb = sm.tile([128, 2], F32, tag="ymu")
             nc.sync.dma_start(out=y[:, :, :], in_=ys.rearrange("(g p) j -> p g j", p=128))
             nc.sync.dma_start(out=x[:, :, :], in_=xs.rearrange("(g p) j -> p g j", p=128))
             nc.sync.dma_start(out=ymu_sb[:, :], in_=ymu[:, :])

             ones_col = sm.tile([128, 1], F32, tag="ones_col")
             nc.vector.memset(ones_col[:, :], 1.0)

             # ---------------- center on DVE (tensor_scalar, 2x) ----------
             # y is centered in place; x centers into xc (f32r).
             for g in range(2):
                 nc.vector.tensor_scalar(
                     out=y[:, g, :], in0=y[:, g, :],
                     scalar1=ymu_sb[:, g : g + 1], scalar2=None, op0=ALU.subtract)
             for g in range(2):
                 nc.vector.tensor_scalar(
                     out=xc[:, g, :], in0=x[:, g, :],
                     scalar1=ymu_sb[:, g : g + 1], scalar2=None, op0=ALU.subtract)

             # ---------------- channel sumsq -> 1/norm (col layout) -------
             # pscol[:, 32+r] (y) first: invy gates yn which gates pass 1.
             pscol = mmq.tile([128, 64], F32, tag="pqA")
             for ti, src in ((1, y), (0, xc)):
                 for ch in range(4):
                     sqs = []
                     for g in range(2):
                         sq = sqp.tile([128, 1024], F32, tag="sqt")
                         nc.scalar.activation(
                             out=sq[:, :],
                             in_=src[:, g, 1024 * ch : 1024 * (ch + 1)].bitcast(F32),
                             func=ACTF.Square)
                         sqs.append(sq)
                     for k in range(8):
                         r = 8 * ch + k
                         for g in range(2):
                             nc.tensor.matmul(
                                 pscol[:, 32 * ti + r : 32 * ti + r + 1],
                                 sqs[g][:, 128 * k : 128 * (k + 1)],
                                 ones_col[:, :],
                                 start=(g == 0), stop=(g == 1))
             # y norms first (critical path), then x
             norms = sm.tile([128, 64], F32, tag="norms")
             invc = sm.tile([128, 64], F32, tag="invc")
             nc.scalar.activation(
                 out=norms[:, 32:64], in_=pscol[:, 32:64], func=ACTF.Sqrt)
             nc.vector.reciprocal(invc[:, 32:64], norms[:, 32:64])
             nc.sync.dma_start(
                 out=scr_y[:, :].rearrange("r p -> p r"), in_=invc[:, 32:64])
             nc.scalar.activation(
                 out=norms[:, 0:32], in_=pscol[:, 0:32], func=ACTF.Sqrt)
             nc.vector.reciprocal(invc[:, 0:32], norms[:, 0:32])
             neginvx = sm.tile([128, 32], F32, tag="neginvx")
             nc.vector.tensor_scalar(
                 out=neginvx[:, :], in0=invc[:, 0:32], scalar1=-1.0,
                 scalar2=None, op0=ALU.mult)
             twoinvx = sm.tile([128, 32], F32, tag="twoinvx")
             nc.vector.tensor_scalar(
                 out=twoinvx[:, :], in0=invc[:, 0:32], scalar1=2.0,
                 scalar2=None, op0=ALU.mult)

             # invy broadcast along partitions, chunked so yn can start early
             invybc = bc3.tile([128, HW], F32, tag="bcast")
             for cc in range(4):
                 bcast_src_y = bass_mod.AP(
                     tensor=scr_y[:, :].tensor, offset=1024 * cc,
                     ap=[[0, 128], [1, 1024]])
                 nc.sync.dma_start(
                     out=invybc[:, 1024 * cc : 1024 * (cc + 1)], in_=bcast_src_y)

             # ---------------- y_n = y_c * invy (f32r), chunked ------------
             for g in range(2):
                 for cc in range(4):
                     sl = slice(1024 * cc, 1024 * (cc + 1))
                     nc.vector.tensor_tensor(
                         out=yn[:, g, sl], in0=y[:, g, sl], in1=invybc[:, sl],
                         op=ALU.mult)

             # dummy bf16 weights: standalone LDWEIGHTS issued during the
             # pass-1 stats gate keep the PE activity monitor from
             # re-throttling the clock (they touch no PSUM and the next
             # real matmul self-loads its own weights anyway).
             wdum = sm.tile([128, 128], BF16, tag="wdum")
             nc.vector.memset(wdum[:, :], 0.0)

             # ---------------- PASS 1: row max + Z -------------------------
             # Four PSUM tiles [128, 2, 512] (2 banks each) per block, so
             # the next block's matmuls chase per-tile frees (exp(q) is
             # the last reader of tile q).  Row-max: FD-1024 for tiles
             # 0-2 (overlap the MM stream), FD-512 x2 for the last tile
             # (short tail).  The softmax u = w/Z is shift-invariant so
             # the exp runs with scale=alpha only; pass 2 biases -lnZ'.
             gacc8 = sm.tile([128, 256], F32, tag="gacc8")
             zacc = sm.tile([128, 128], F32, tag="zacc")
             gmaxc = sm.tile([128, 32], F32, tag="gmaxc")
             reccol = sm.tile([128, 32], F32, tag="reccol")
             acol = sm.tile([128, 32], F32, tag="acol")
             tmpc = sm.tile([128, 32], F32, tag="tmpc")

             CH = ((0,), (1, 2), (3, 4), (5, 6, 7))  # chunks per tile
             for r in range(NB):
                 ts = [mmq.tile([128, len(ch), 512], F32, tag=f"pq{'ABCD'[q]}",
                                name=f"pq_{r}_{q}")
                       for q, ch in enumerate(CH)]
                 for q, ch in enumerate(CH):
                     for g in range(2):
                         for j, c in enumerate(ch):
                             nc.tensor.matmul(
                                 ts[q][:, j, :],
                                 xc[:, g, 128 * r : 128 * (r + 1)],
                                 yn[:, g, 512 * c : 512 * (c + 1)],
                                 start=(g == 0), stop=(g == 1))
                     if q < 3:
                         nc.vector.reduce_max(
                             gacc8[:, 8 * r + q : 8 * r + q + 1],
                             ts[q][:, :, :], axis=AX.XY)
                     else:
                         nc.vector.reduce_max(
                             gacc8[:, 8 * r + 3 : 8 * r + 4],
                             ts[3][:, 0:2, :], axis=AX.XY)
                         nc.vector.reduce_max(
                             gacc8[:, 8 * r + 4 : 8 * r + 5],
                             ts[3][:, 2, :], axis=AX.X)
                 # keep the PE busy through the stats gate (HAM warm)
                 for _ in range(12):
                     nc.tensor.ldweights(wdum[:, :])
                 # alpha chain (no b1: softmax shift-invariance)
                 nc.vector.reduce_max(
                     gmaxc[:, r : r + 1],
                     gacc8[:, 8 * r : 8 * r + 5], axis=AX.X)
                 nc.vector.tensor_scalar(
                     out=tmpc[:, r : r + 1], in0=gmaxc[:, r : r + 1],
                     scalar1=neginvx[:, r : r + 1], scalar2=float(1.0 + EPS),
                     op0=ALU.mult, op1=ALU.add)
                 nc.vector.reciprocal(reccol[:, r : r + 1], tmpc[:, r : r + 1])
                 nc.vector.tensor_scalar(
                     out=acol[:, r : r + 1], in0=reccol[:, r : r + 1],
                     scalar1=twoinvx[:, r : r + 1], scalar2=None, op0=ALU.mult)
                 for q in range(4):
                     nc.scalar.activation(
                         out=ts[q][:, :, :], in_=ts[q][:, :, :],
                         func=ACTF.Exp, scale=acol[:, r : r + 1],
                         accum_out=zacc[:, 4 * r + q : 4 * r + q + 1])

             # ---------------- interlude: b2 = -lnZ; broadcasts ------------
             zsum = sm.tile([128, 32], F32, tag="zsum")
             nc.vector.reduce_sum(
                 zsum[:, :], zacc[:, :].rearrange("p (r q) -> p r q", q=4),
                 axis=AX.X)
             lnz = sm.tile([128, 32], F32, tag="lnz")
             nc.scalar.activation(out=lnz[:, :], in_=zsum[:, :], func=ACTF.Ln)
             b2c = sm.tile([128, 32], F32, tag="b2c")
             nc.vector.tensor_scalar(
                 out=b2c[:, :], in0=lnz[:, :], scalar1=-1.0,
                 scalar2=None, op0=ALU.mult)

             # alpha cols -> DRAM rows -> part-bcast; b2 cols -> one row
             nc.sync.dma_start(
                 out=scr_a[:, :].rearrange("r p -> p r"), in_=acol[:, :])
             nc.sync.dma_start(
                 out=scr_b[:, :].rearrange("r p -> p r"), in_=b2c[:, :])
             abc = bc3.tile([128, HW], F32, tag="bcast")
             for cc in range(4):
                 bcast_src_a = bass_mod.AP(
                     tensor=scr_a[:, :].tensor, offset=1024 * cc,
                     ap=[[0, 128], [1, 1024]])
                 nc.sync.dma_start(
                     out=abc[:, 1024 * cc : 1024 * (cc + 1)], in_=bcast_src_a)
             b2row_f = sm.tile([1, HW], F32, tag="b2row_f")
             nc.sync.dma_start(
                 out=b2row_f[0:1, :],
                 in_=scr_b[:, :].rearrange("r p -> (r p)"))
             b2row = sm.tile([1, HW], F32R, tag="b2row")
             nc.vector.tensor_scalar(
                 out=b2row[:, :], in0=b2row_f[:, :], scalar1=1.0, scalar2=None,
                 op0=ALU.mult)
             ones_row_f = sm.tile([1, 128], F32, tag="ones_row_f")
             nc.vector.memset(ones_row_f[:, :], 1.0)
             ones_row_r = sm.tile([1, 128], F32R, tag="ones_row_r")
             nc.vector.tensor_scalar(
                 out=ones_row_r[:, :], in0=ones_row_f[:, :], scalar1=1.0,
                 scalar2=None, op0=ALU.mult)

             # x2 = x_c * alpha  (in place, f32r), chunked
             for g in range(2):
                 for cc in range(4):
                     sl = slice(1024 * cc, 1024 * (cc + 1))
                     nc.vector.tensor_tensor(
                         out=xc[:, g, sl], in0=xc[:, g, sl].bitcast(F32),
                         in1=abc[:, sl], op=ALU.mult)

             # ---------------- PASS 2: col max of A ------------------------
             macc = sm.tile([128, 128], F32, tag="macc")
             for rb in range(NB):
                 ts = [mmq.tile([128, len(ch), 512], F32, tag=f"pq{'ABCD'[q]}",
                                name=f"pq2_{rb}_{q}")
                       for q, ch in enumerate(CH)]
                 for q, ch in enumerate(CH):
                     for j, c in enumerate(ch):
                         nc.tensor.matmul(
                             ts[q][:, j, :],
                             ones_row_r[:, :],
                             b2row[:, 512 * c : 512 * (c + 1)],
                             start=True, stop=False)
                 for q, ch in enumerate(CH):
                     for g in range(2):
                         for j, c in enumerate(ch):
                             nc.tensor.matmul(
                                 ts[q][:, j, :],
                                 yn[:, g, 128 * rb : 128 * (rb + 1)],
                                 xc[:, g, 512 * c : 512 * (c + 1)],
                                 start=False, stop=(g == 1))
                     nc.vector.reduce_max(
                         macc[:, 4 * rb + q : 4 * rb + q + 1],
                         ts[q][:, :, :], axis=AX.XY)

             # ---------------- final ---------------------------------------
             mcol = sm.tile([128, 32], F32, tag="mcol")
             nc.vector.reduce_max(
                 mcol[:, :], macc[:, :].rearrange("p (r q) -> p r q", q=4),
                 axis=AX.X)
             expm = sm.tile([128, 32], F32, tag="expm")
             csum = sm.tile([128, 1], F32, tag="csum")
             nc.scalar.activation(
                 out=expm[:, :], in_=mcol[:, :], func=ACTF.Exp,
                 accum_out=csum[:, :])
             tot = sm.tile([128, 1], F32, tag="tot")
             nc.gpsimd.partition_all_reduce(
                 tot[:, :], csum[:, :], channels=128,
                 reduce_op=bass_isa.ReduceOp.add)
             res = sm.tile([1, 1], F32, tag="res")
             nc.vector.tensor_scalar(
                 out=res[:, :], in0=tot[0:1, :], scalar1=float(1.0 / HW),
                 scalar2=None, op0=ALU.mult)
             nc.sync.dma_start(out=out[:, :], in_=res[:, :])
    nc.compile()
    return nc


def _get_nc():
    if "nc" not in _cached:
        _cached["nc"] = _build()
    return _cached["nc"]


def run_device(x, y, trace=False):
    """x, y: (N, C, H, W) float32. Returns (ccx (N,), BassKernelResults)."""
    x = np.ascontiguousarray(np.asarray(x, dtype=np.float32))
    y = np.ascontiguousarray(np.asarray(y, dtype=np.float32))
    ymu = y.mean(axis=(0, 2, 3), dtype=np.float64).astype(np.float32)  # (C,)
    ymu_arr = np.ascontiguousarray(ymu.reshape(2, 128).T)  # (128, 2)
    in_maps = []
    for n in range(N):
        in_maps.append({
            "xs": np.ascontiguousarray(x[n].reshape(C, HW)),
            "ys": np.ascontiguousarray(y[n].reshape(C, HW)),
            "ymu": ymu_arr,
        })
    nc = _get_nc()
    res = run_bass_kernel_spmd(nc, in_maps, core_ids=list(range(N)), trace=trace)
    ccx = np.array([res.results[n]["out"][0, 0] for n in range(N)], dtype=np.float32)
    return ccx, res


def kernel(x, y):
    ccx, _ = run_device(x, y)
    loss = float(np.mean(-np.log(ccx.astype(np.float64) + EPS)))
    return np.float32(loss)


if __name__ == "__main__":
    rng = np.random.default_rng(0)
    x = rng.standard_normal((N, C, H, W), dtype=np.float32)
    y = rng.standard_normal((N, C, H, W), dtype=np.float32)
    print("loss:", kernel(x, y))



# revision 2
# speedup vs baseline: 1.5819x; 1.5819x over previous
"""CCX loss kernel for Trainium2 (8 NeuronCores, data-parallel over batch).

Math (per batch element n, C=256 channels, HW=64*64=4096 pixels):
  y_mu[c]   = mean over (n, h, w) of y            (host, tiny)
  x_c = x - y_mu ; y_c = y - y_mu
  x_n = x_c/||x_c||_C ; y_n = y_c/||y_c||_C
  s[i,j]    = sum_c x_n[c,i] y_n[c,j]
  d = 1-s ; dt = d/(dmin_i+eps) ; w = exp((1-dt)/0.5)
  ccx_ij = w/sum_j w ; ccx_n = mean_j max_i ccx_ij
  loss = mean_n -log(ccx_n + eps)                 (host, 8 scalars)

Device identities:
  u_ij = exp(alpha_i*G_ij) / Z_i   with  G = xc^T yn (unnormalized-x),
     alpha_i = 2*invx_i/(dmin_i+eps),  Z_i = sum_j exp(alpha_i G_ij)
  ccx_n = mean_j max_i u_ij

Schedule (per 128-row block r of G):
  A-stream: fp8e4 DoubleRow matmuls (full K=256 in one matmul) compute
    G quarters in PSUM (2 banks, double buffered) -> DVE rowmax -> free.
    Stats chain gives alpha_r.
  B-stream: regenerate the same G quarters (PE is cheap: 256 cyc per
    512-col DR matmul) -> Act exp(scale=alpha_r) writes E_r to SBUF
    bf16 with accum -> Z_r.
  ACC update (one fused DVE op, all-SBUF bf16):
    ACC = max(ACC, E_r * (1/Z_r))      [scalar_tensor_tensor]
  Finally 32 PE transposes of ACC give the cross-partition (over i)
  column max via cheap [128,128] reduces; ccx = mean_j ACC-max.

Accuracy: fp8e4m3 inputs to the similarity matmul, bf16 E/U. CPU
simulation vs the fp32 reference gives rel err ~2e-3 (gate is 2e-2).
"""

import os
import sys

import numpy as np

sys.path.insert(0, "/opt/trn_rl_repo")
os.environ.setdefault("JAX_PLATFORMS", "axon")

import concourse.mybir as mybir
import concourse.tile as tile
from concourse import bacc, bass_isa
from concourse.bass_utils import run_bass_kernel_spmd
from concourse.masks import make_identity

N, C, H, W = 8, 256, 64, 64
HW = H * W          # 4096
NB = HW // 128      # 32 blocks of 128 rows
EPS = 1e-6
F32 = mybir.dt.float32
BF16 = mybir.dt.bfloat16
FP8 = mybir.dt.float8e4
ALU = mybir.AluOpType
ACTF = mybir.ActivationFunctionType
AX = mybir.AxisListType
DR = mybir.MatmulPerfMode.DoubleRow

_cached = {}


def _build():
    nc = bacc.Bacc(None, target_bir_lowering=False, debug=True)
    xs = nc.dram_tensor("xs", [C, HW], F32, kind="ExternalInput")
    ys = nc.dram_tensor("ys", [C, HW], F32, kind="ExternalInput")
    nmu = nc.dram_tensor("nmu", [128, 2], F32, kind="ExternalInput")  # -mean
    out = nc.dram_tensor("out", [1, 1], F32, kind="ExternalOutput")
    scr_y = nc.dram_tensor("scr_y", [NB, 128], F32)

    import concourse.bass as bass_mod

    with tile.TileContext(nc) as tc:
        with (
            tc.tile_pool(name="big", bufs=1) as big,
            tc.tile_pool(name="bc", bufs=1) as bc,
            tc.tile_pool(name="sq", bufs=2) as sqp,
            tc.tile_pool(name="eb", bufs=2) as ebp,
            tc.tile_pool(name="sm", bufs=1) as sm,
            tc.tile_pool(name="mmq", bufs=1, space="PSUM") as mmq,
        ):
            # ---------------- load ----------------
            x = big.tile([128, 2, HW], F32, tag="x")
            y = big.tile([128, 2, HW], F32, tag="y")
            xc8 = big.tile([128, 2, HW], FP8, tag="xc8")
            yn8 = big.tile([128, 2, HW], FP8, tag="yn8")
            acc = big.tile([128, HW], BF16, tag="acc")
            nmu_sb = sm.tile([128, 2], F32, tag="nmu")
            nc.sync.dma_start(out=y[:, :, :], in_=ys.rearrange("(g p) j -> p g j", p=128))
            nc.sync.dma_start(out=nmu_sb[:, :], in_=nmu[:, :])
            nc.sync.dma_start(out=x[:, :, :], in_=xs.rearrange("(g p) j -> p g j", p=128))

            ones_col = sm.tile([128, 1], F32, tag="ones_col")
            nc.vector.memset(ones_col[:, :], 1.0)
            nc.vector.memset(acc[:, :], 0.0)

            # ---------------- channel sumsq -> 1/norm (col layout) -------
            # sq = (t - mu)^2 via Act Square with per-partition bias; the
            # ones-column matmuls contract over channels.  y first (invy
            # gates yn8 which gates everything).
            pscol = mmq.tile([128, 2, 512], F32, tag="qa0")
            for ti, src in ((1, y), (0, x)):
                for ch in range(4):
                    sqs = []
                    for g in range(2):
                        sq = sqp.tile([128, 1024], F32, tag="sqt")
                        nc.scalar.activation(
                            out=sq[:, :],
                            in_=src[:, g, 1024 * ch : 1024 * (ch + 1)],
                            func=ACTF.Square, bias=nmu_sb[:, g : g + 1])
                        sqs.append(sq)
                    for k in range(8):
                        r = 8 * ch + k
                        for g in range(2):
                            nc.tensor.matmul(
                                pscol[:, ti, r : r + 1],
                                sqs[g][:, 128 * k : 128 * (k + 1)],
                                ones_col[:, :],
                                start=(g == 0), stop=(g == 1))
            norms = sm.tile([128, 64], F32, tag="norms")
            invc = sm.tile([128, 64], F32, tag="invc")
            nc.scalar.activation(
                out=norms[:, 32:64], in_=pscol[:, 1, 0:32], func=ACTF.Sqrt)
            nc.vector.reciprocal(invc[:, 32:64], norms[:, 32:64])
            nc.sync.dma_start(
                out=scr_y[:, :].rearrange("r p -> p r"), in_=invc[:, 32:64])
            nc.scalar.activation(
                out=norms[:, 0:32], in_=pscol[:, 0, 0:32], func=ACTF.Sqrt)
            nc.vector.reciprocal(invc[:, 0:32], norms[:, 0:32])
            neginvx = sm.tile([128, 32], F32, tag="neginvx")
            nc.vector.tensor_scalar(
                out=neginvx[:, :], in0=invc[:, 0:32], scalar1=-1.0,
                scalar2=None, op0=ALU.mult)
            twoinvx = sm.tile([128, 32], F32, tag="twoinvx")
            nc.vector.tensor_scalar(
                out=twoinvx[:, :], in0=invc[:, 0:32], scalar1=2.0,
                scalar2=None, op0=ALU.mult)

            # invy broadcast along partitions (DRAM bounce)
            invybc = bc.tile([128, HW], F32, tag="invybc")
            for cc in range(4):
                bcast_src_y = bass_mod.AP(
                    tensor=scr_y[:, :].tensor, offset=1024 * cc,
                    ap=[[0, 128], [1, 1024]])
                nc.sync.dma_start(
                    out=invybc[:, 1024 * cc : 1024 * (cc + 1)], in_=bcast_src_y)

            # ---------------- fp8 operands ----------------
            # yn8 = (y - mu) * invy  (one fused STT per channel group)
            for g in range(2):
                nc.vector.scalar_tensor_tensor(
                    out=yn8[:, g, :], in0=y[:, g, :],
                    scalar=nmu_sb[:, g : g + 1], in1=invybc[:, :],
                    op0=ALU.add, op1=ALU.mult)
            # xc8 = x - mu
            for g in range(2):
                nc.vector.tensor_scalar(
                    out=xc8[:, g, :], in0=x[:, g, :],
                    scalar1=nmu_sb[:, g : g + 1], scalar2=None, op0=ALU.add)

            # ---------------- main loop over row blocks -------------------
            gacc = sm.tile([128, 128], F32, tag="gacc")
            zacc = sm.tile([128, 128], F32, tag="zacc")
            gmaxc = sm.tile([128, 32], F32, tag="gmaxc")
            tmpc = sm.tile([128, 32], F32, tag="tmpc")
            reccol = sm.tile([128, 32], F32, tag="reccol")
            acol = sm.tile([128, 32], F32, tag="acol")
            zsum = sm.tile([128, 32], F32, tag="zsum")
            zinv = sm.tile([128, 32], F32, tag="zinv")
            ebufs = {}

            def tail_ops(r):
                # Z_r, 1/Z_r, ACC = max(ACC, E_r/Z_r); emitted one row late
                # so the in-order DVE queue never waits on Act.
                nc.vector.reduce_sum(
                    zsum[:, r : r + 1],
                    zacc[:, 4 * r : 4 * r + 4], axis=AX.X)
                nc.vector.reciprocal(zinv[:, r : r + 1], zsum[:, r : r + 1])
                nc.vector.scalar_tensor_tensor(
                    out=acc[:, :], in0=ebufs[r % 2][:, :],
                    scalar=zinv[:, r : r + 1], in1=acc[:, :],
                    op0=ALU.mult, op1=ALU.max)

            for r in range(NB):
                lhs = xc8[:, :, 128 * r : 128 * (r + 1)]
                # ---- A-stream: G quarters + rowmax ----
                for q in range(4):
                    qa = mmq.tile([128, 2, 512], F32, tag=f"qa{q % 2}",
                                  name=f"qa_{r}_{q}")
                    for cc in range(2):
                        c = 2 * q + cc
                        nc.tensor.matmul(
                            qa[:, cc, :], lhs,
                            yn8[:, :, 512 * c : 512 * (c + 1)],
                            start=True, stop=True, perf_mode=DR)
                    nc.vector.reduce_max(
                        gacc[:, 4 * r + q : 4 * r + q + 1],
                        qa[:, :, :], axis=AX.XY)
                # ---- stats chain -> alpha_r ----
                nc.vector.reduce_max(
                    gmaxc[:, r : r + 1], gacc[:, 4 * r : 4 * r + 4], axis=AX.X)
                nc.vector.tensor_scalar(
                    out=tmpc[:, r : r + 1], in0=gmaxc[:, r : r + 1],
                    scalar1=neginvx[:, r : r + 1], scalar2=float(1.0 + EPS),
                    op0=ALU.mult, op1=ALU.add)
                nc.vector.reciprocal(reccol[:, r : r + 1], tmpc[:, r : r + 1])
                nc.vector.tensor_scalar(
                    out=acol[:, r : r + 1], in0=reccol[:, r : r + 1],
                    scalar1=twoinvx[:, r : r + 1], scalar2=None, op0=ALU.mult)
                # ---- B-stream: regen + exp -> E_r (bf16), Z accum ----
                eb = ebp.tile([128, HW], BF16, tag=f"eb{r % 2}",
                              name=f"eb_{r}")
                ebufs[r % 2] = eb
                for h in range(4):
                    qb = mmq.tile([128, 2, 512], F32, tag=f"qb{h % 2}",
                                  name=f"qb_{r}_{h}")
                    for cc in range(2):
                        c = 2 * h + cc
                        nc.tensor.matmul(
                            qb[:, cc, :], lhs,
                            yn8[:, :, 512 * c : 512 * (c + 1)],
                            start=True, stop=True, perf_mode=DR)
                    nc.scalar.activation(
                        out=eb[:, 1024 * h : 1024 * (h + 1)], in_=qb[:, :, :],
                        func=ACTF.Exp, scale=acol[:, r : r + 1],
                        accum_out=zacc[:, 4 * r + h : 4 * r + h + 1])
                if r > 0:
                    tail_ops(r - 1)
            tail_ops(NB - 1)

            # ---------------- final: cross-partition max of ACC -----------
            ident = sm.tile([128, 128], BF16, tag="ident")
            make_identity(nc, ident)
            mcol = sm.tile([128, 32], F32, tag="mcol")
            for jb in range(NB):
                tag = ("qa0", "qa1", "qb0", "qb1")[jb % 4]
                tp = mmq.tile([128, 2, 512], F32, tag=tag, name=f"tp_{jb}")
                tpb = tp[:, 0, 0:64].bitcast(BF16)
                nc.tensor.transpose(
                    tpb[:, 0:128], acc[:, 128 * jb : 128 * (jb + 1)],
                    ident[:, :])
                nc.vector.reduce_max(
                    mcol[:, jb : jb + 1], tpb[:, 0:128], axis=AX.X)
            msum = sm.tile([128, 1], F32, tag="msum")
            nc.vector.reduce_sum(msum[:, :], mcol[:, :], axis=AX.X)
            tot = sm.tile([128, 1], F32, tag="tot")
            nc.gpsimd.partition_all_reduce(
                tot[:, :], msum[:, :], channels=128,
                reduce_op=bass_isa.ReduceOp.add)
            res = sm.tile([1, 1], F32, tag="res")
            nc.vector.tensor_scalar(
                out=res[:, :], in0=tot[0:1, :], scalar1=float(1.0 / HW),
                scalar2=None, op0=ALU.mult)
            nc.sync.dma_start(out=out[:, :], in_=res[:, :])
    nc.compile()
    return nc


def _get_nc():
    if "nc" not in _cached:
        _cached["nc"] = _build()
    return _cached["nc"]


def run_device(x, y, trace=False):
    """x, y: (N, C, H, W) float32. Returns (ccx (N,), BassKernelResults)."""
    x = np.ascontiguousarray(np.asarray(x, dtype=np.float32))
    y = np.ascontiguousarray(np.asarray(y, dtype=np.float32))
    ymu = y.mean(axis=(0, 2, 3), dtype=np.float64).astype(np.float32)  # (C,)
    nmu_arr = np.ascontiguousarray((-ymu).reshape(2, 128).T)  # (128, 2)
    in_maps = []
    for n in range(N):
        in_maps.append({
            "xs": np.ascontiguousarray(x[n].reshape(C, HW)),
            "ys": np.ascontiguousarray(y[n].reshape(C, HW)),
            "nmu": nmu_arr,
        })
    nc = _get_nc()
    res = run_bass_kernel_spmd(nc, in_maps, core_ids=list(range(N)), trace=trace)
    ccx = np.array([res.results[n]["out"][0, 0] for n in range(N)], dtype=np.float32)
    return ccx, res


def kernel(x, y):
    ccx, _ = run_device(x, y)
    loss = float(np.mean(-np.log(ccx.astype(np.float64) + EPS)))
    return np.float32(loss)


if __name__ == "__main__":
    rng = np.random.default_rng(0)
    x = rng.standard_normal((N, C, H, W), dtype=np.float32)
    y = rng.standard_normal((N, C, H, W), dtype=np.float32)
    print("loss:", kernel(x, y))


# revision 11
# speedup vs baseline: 1.7481x; 1.1050x over previous
"""CCX loss kernel for Trainium2 (8 NeuronCores, data-parallel over batch).

Math (per batch element n, C=256 channels, HW=64*64=4096 pixels):
  y_mu[c]   = mean over (n, h, w) of y            (host, tiny)
  x_c = x - y_mu ; y_c = y - y_mu
  x_n = x_c/||x_c||_C ; y_n = y_c/||y_c||_C
  s[i,j]    = sum_c x_n[c,i] y_n[c,j]
  d = 1-s ; dt = d/(dmin_i+eps) ; w = exp((1-dt)/0.5)
  ccx_ij = w/sum_j w ; ccx_n = mean_j max_i ccx_ij
  loss = mean_n -log(ccx_n + eps)                 (host, 8 scalars)

Device identities:
  u_ij = exp(alpha_i*G_ij) / Z_i   with  G = xc^T yn (unnormalized-x),
     alpha_i = 2*invx_i/(dmin_i+eps),  Z_i = sum_j exp(alpha_i G_ij)
  ccx_n = mean_j max_i u_ij

Schedule (per 128-row block r of G):
  A-stream: fp8e4 DoubleRow matmuls (full K=256 in one matmul) compute
    G quarters in PSUM (2 banks, double buffered) -> DVE rowmax -> free.
    Stats chain gives alpha_r.
  B-stream: regenerate the same G quarters (PE is cheap: 256 cyc per
    512-col DR matmul) -> Act exp(scale=alpha_r) writes E_r to SBUF
    bf16 with accum -> Z_r.
  ACC update (one fused DVE op, all-SBUF bf16):
    ACC = max(ACC, E_r * (1/Z_r))      [scalar_tensor_tensor]
  Finally 32 PE transposes of ACC give the cross-partition (over i)
  column max via cheap [128,128] reduces; ccx = mean_j ACC-max.

Accuracy: fp8e4m3 inputs to the similarity matmul, bf16 E/U. CPU
simulation vs the fp32 reference gives rel err ~2e-3 (gate is 2e-2).
"""

import os
import sys

import numpy as np

sys.path.insert(0, "/opt/trn_rl_repo")
os.environ.setdefault("JAX_PLATFORMS", "axon")

import concourse.mybir as mybir
import concourse.tile as tile
from concourse import bacc, bass_isa
from concourse import bass_utils as _bu
from concourse.bass_utils import run_bass_kernel_spmd
from concourse.masks import make_identity



N, C, H, W = 8, 256, 64, 64
HW = H * W          # 4096
NB = HW // 128      # 32 blocks of 128 rows
EPS = 1e-6
F32 = mybir.dt.float32
BF16 = mybir.dt.bfloat16
FP8 = mybir.dt.float8e4
ALU = mybir.AluOpType
ACTF = mybir.ActivationFunctionType
AX = mybir.AxisListType
DR = mybir.MatmulPerfMode.DoubleRow

_cached = {}


def _build():
    nc = bacc.Bacc(None, target_bir_lowering=False, debug=True)
    xs = nc.dram_tensor("xs", [C, HW], F32, kind="ExternalInput")
    ys = nc.dram_tensor("ys", [C, HW], F32, kind="ExternalInput")
    nmu = nc.dram_tensor("nmu", [128, 2], F32, kind="ExternalInput")  # -mean
    out = nc.dram_tensor("out", [1, 1], F32, kind="ExternalOutput")
    scr_y = nc.dram_tensor("scr_y", [NB, 128], F32)
    scr_n = nc.dram_tensor("scr_n", [2, HW], F32)

    import concourse.bass as bass_mod

    with tile.TileContext(nc) as tc:
        with (
            tc.tile_pool(name="big", bufs=1) as big,
            tc.tile_pool(name="bc", bufs=1) as bc,
            tc.tile_pool(name="sq", bufs=2) as sqp,
            tc.tile_pool(name="eb", bufs=2) as ebp,
            tc.tile_pool(name="sm", bufs=1) as sm,
            tc.tile_pool(name="mmq", bufs=1, space="PSUM") as mmq,
        ):
            # ---------------- load ----------------
            x = big.tile([128, 2, HW], F32, tag="x")
            y = big.tile([128, 2, HW], F32, tag="y")
            xc8 = big.tile([128, 2, HW], FP8, tag="xc8")
            yn8 = big.tile([128, 2, HW], FP8, tag="yn8")
            acc = big.tile([128, HW], BF16, tag="acc")
            nmu_sb = sm.tile([128, 2], F32, tag="nmu")
            nc.sync.dma_start(out=y[:, :, :], in_=ys.rearrange("(g p) j -> p g j", p=128))
            nc.sync.dma_start(out=nmu_sb[:, :], in_=nmu[:, :])
            nc.sync.dma_start(out=x[:, :, :], in_=xs.rearrange("(g p) j -> p g j", p=128))

            ones_col = sm.tile([128, 1], BF16, tag="ones_col")
            nc.vector.memset(ones_col[:, :], 1.0)
            nc.vector.memset(acc[:, :], 0.0)

            # ---------------- channel sumsq -> 1/norm --------------------
            # sq = (t - mu)^2 via Act Square (bf16) with per-partition
            # bias; ones-STATIONARY matmuls (trivial weight loads)
            # contract the channel groups into PSUM row slices [1, 512],
            # which bounce through DRAM into column layout.  y first
            # (invy gates yn8 which gates everything).
            for ti, src in ((1, y), (0, x)):
                rowbuf = sm.tile([1, HW], F32, tag=f"rowbuf{ti}")
                for ch in range(4):
                    sq = sqp.tile([128, 2, 1024], BF16, tag="sqt")
                    for g in range(2):
                        nc.scalar.activation(
                            out=sq[:, g, :],
                            in_=src[:, g, 1024 * ch : 1024 * (ch + 1)],
                            func=ACTF.Square, bias=nmu_sb[:, g : g + 1])
                    pt = mmq.tile([128, 2, 512], F32, tag=f"qa{ch % 2}",
                                  name=f"nsq_{ti}_{ch}")
                    for cc in range(2):
                        for g in range(2):
                            nc.tensor.matmul(
                                pt[0:1, cc, :],
                                ones_col[:, :],
                                sq[:, g, 512 * cc : 512 * (cc + 1)],
                                start=(g == 0), stop=(g == 1))
                    nc.scalar.copy(
                        rowbuf[0:1, 1024 * ch : 1024 * (ch + 1)],
                        pt[0:1, :, :])
                nc.sync.dma_start(out=scr_n[ti : ti + 1, :], in_=rowbuf[:, :])
            nsq = sm.tile([128, 64], F32, tag="nsq")
            norms = sm.tile([128, 64], F32, tag="norms")
            invc = sm.tile([128, 64], F32, tag="invc")
            nc.sync.dma_start(
                out=nsq[:, 32:64],
                in_=scr_n[1, :].rearrange("(r p) -> p r", p=128))
            nc.scalar.activation(
                out=norms[:, 32:64], in_=nsq[:, 32:64], func=ACTF.Sqrt)
            nc.vector.reciprocal(invc[:, 32:64], norms[:, 32:64])
            nc.sync.dma_start(
                out=scr_y[:, :].rearrange("r p -> p r"), in_=invc[:, 32:64])
            nc.sync.dma_start(
                out=nsq[:, 0:32],
                in_=scr_n[0, :].rearrange("(r p) -> p r", p=128))
            nc.scalar.activation(
                out=norms[:, 0:32], in_=nsq[:, 0:32], func=ACTF.Sqrt)
            nc.vector.reciprocal(invc[:, 0:32], norms[:, 0:32])
            neginvx = sm.tile([128, 32], F32, tag="neginvx")
            nc.vector.tensor_scalar(
                out=neginvx[:, :], in0=invc[:, 0:32], scalar1=-1.0,
                scalar2=None, op0=ALU.mult)
            twoinvx = sm.tile([128, 32], F32, tag="twoinvx")
            nc.vector.tensor_scalar(
                out=twoinvx[:, :], in0=invc[:, 0:32], scalar1=2.0,
                scalar2=None, op0=ALU.mult)

            # invy broadcast along partitions (DRAM bounce)
            invybc = bc.tile([128, HW], F32, tag="invybc")
            for cc in range(4):
                bcast_src_y = bass_mod.AP(
                    tensor=scr_y[:, :].tensor, offset=1024 * cc,
                    ap=[[0, 128], [1, 1024]])
                nc.sync.dma_start(
                    out=invybc[:, 1024 * cc : 1024 * (cc + 1)], in_=bcast_src_y)

            # ---------------- fp8 operands ----------------
            # yn8 = (y - mu) * invy  (one fused STT per channel group)
            for g in range(2):
                nc.vector.scalar_tensor_tensor(
                    out=yn8[:, g, :], in0=y[:, g, :],
                    scalar=nmu_sb[:, g : g + 1], in1=invybc[:, :],
                    op0=ALU.add, op1=ALU.mult)
            # xc8 = x - mu (Act: per-partition bias, fp8 out; keeps DVE free)
            for g in range(2):
                nc.scalar.activation(
                    out=xc8[:, g, :], in_=x[:, g, :],
                    func=ACTF.Identity, bias=nmu_sb[:, g : g + 1])

            # ---------------- main loop over row blocks -------------------
            gacc = sm.tile([128, 128], F32, tag="gacc")
            zacc = sm.tile([128, 128], F32, tag="zacc")
            gmaxc = sm.tile([128, 32], F32, tag="gmaxc")
            tmpc = sm.tile([128, 32], F32, tag="tmpc")
            reccol = sm.tile([128, 32], F32, tag="reccol")
            acol = sm.tile([128, 32], F32, tag="acol")
            zsum = sm.tile([128, 32], F32, tag="zsum")
            zinv = sm.tile([128, 32], F32, tag="zinv")
            ebufs = {}

            def tail_ops(r):
                # Z_r, 1/Z_r, ACC = max(ACC, E_r/Z_r); emitted one row late
                # so the in-order DVE queue never waits on Act.
                nc.vector.reduce_sum(
                    zsum[:, r : r + 1],
                    zacc[:, 4 * r : 4 * r + 4], axis=AX.X)
                nc.vector.reciprocal(zinv[:, r : r + 1], zsum[:, r : r + 1])
                nc.vector.scalar_tensor_tensor(
                    out=acc[:, :], in0=ebufs[r % 2][:, :],
                    scalar=zinv[:, r : r + 1], in1=acc[:, :],
                    op0=ALU.mult, op1=ALU.max)

            for r in range(NB):
                lhs = xc8[:, :, 128 * r : 128 * (r + 1)]
                # ---- A-stream: G quarters + rowmax ----
                for q in range(4):
                    qa = mmq.tile([128, 2, 512], F32, tag=f"qa{q % 2}",
                                  name=f"qa_{r}_{q}")
                    for cc in range(2):
                        c = 2 * q + cc
                        nc.tensor.matmul(
                            qa[:, cc, :], lhs,
                            yn8[:, :, 512 * c : 512 * (c + 1)],
                            start=True, stop=True, perf_mode=DR)
                    nc.vector.reduce_max(
                        gacc[:, 4 * r + q : 4 * r + q + 1],
                        qa[:, :, :], axis=AX.XY)
                # ---- stats chain -> alpha_r ----
                nc.vector.reduce_max(
                    gmaxc[:, r : r + 1], gacc[:, 4 * r : 4 * r + 4], axis=AX.X)
                nc.vector.tensor_scalar(
                    out=tmpc[:, r : r + 1], in0=gmaxc[:, r : r + 1],
                    scalar1=neginvx[:, r : r + 1], scalar2=float(1.0 + EPS),
                    op0=ALU.mult, op1=ALU.add)
                nc.vector.reciprocal(reccol[:, r : r + 1], tmpc[:, r : r + 1])
                nc.vector.tensor_scalar(
                    out=acol[:, r : r + 1], in0=reccol[:, r : r + 1],
                    scalar1=twoinvx[:, r : r + 1], scalar2=None, op0=ALU.mult)
                # ---- B-stream: regen + exp -> E_r (bf16), Z accum ----
                eb = ebp.tile([128, HW], BF16, tag=f"eb{r % 2}",
                              name=f"eb_{r}")
                ebufs[r % 2] = eb
                for h in range(4):
                    qb = mmq.tile([128, 2, 512], F32, tag=f"qb{h % 2}",
                                  name=f"qb_{r}_{h}")
                    for cc in range(2):
                        c = 2 * h + cc
                        nc.tensor.matmul(
                            qb[:, cc, :], lhs,
                            yn8[:, :, 512 * c : 512 * (c + 1)],
                            start=True, stop=True, perf_mode=DR)
                    nc.scalar.activation(
                        out=eb[:, 1024 * h : 1024 * (h + 1)], in_=qb[:, :, :],
                        func=ACTF.Exp, scale=acol[:, r : r + 1],
                        accum_out=zacc[:, 4 * r + h : 4 * r + h + 1])
                if r > 0:
                    tail_ops(r - 1)
            tail_ops(NB - 1)

            # ---------------- final: cross-partition max of ACC -----------
            ident = sm.tile([128, 128], BF16, tag="ident")
            make_identity(nc, ident)
            mcol = sm.tile([128, 32], F32, tag="mcol")
            for jb in range(NB):
                tag = ("qa0", "qa1", "qb0", "qb1")[jb % 4]
                tp = mmq.tile([128, 2, 512], F32, tag=tag, name=f"tp_{jb}")
                tpb = tp[:, 0, 0:64].bitcast(BF16)
                nc.tensor.transpose(
                    tpb[:, 0:128], acc[:, 128 * jb : 128 * (jb + 1)],
                    ident[:, :])
                nc.vector.reduce_max(
                    mcol[:, jb : jb + 1], tpb[:, 0:128], axis=AX.X)
            msum = sm.tile([128, 1], F32, tag="msum")
            nc.vector.reduce_sum(msum[:, :], mcol[:, :], axis=AX.X)
            tot = sm.tile([128, 1], F32, tag="tot")
            nc.gpsimd.partition_all_reduce(
                tot[:, :], msum[:, :], channels=128,
                reduce_op=bass_isa.ReduceOp.add)
            res = sm.tile([1, 1], F32, tag="res")
            nc.vector.tensor_scalar(
                out=res[:, :], in0=tot[0:1, :], scalar1=float(1.0 / HW),
                scalar2=None, op0=ALU.mult)
            nc.sync.dma_start(out=out[:, :], in_=res[:, :])
    nc.compile()
    return nc


def _get_nc():
    if "nc" not in _cached:
        _cached["nc"] = _build()
    return _cached["nc"]


def run_device(x, y, trace=False):
    """x, y: (N, C, H, W) float32. Returns (ccx (N,), BassKernelResults)."""
    x = np.ascontiguousarray(np.asarray(x, dtype=np.float32))
    y = np.ascontiguousarray(np.asarray(y, dtype=np.float32))
    ymu = y.mean(axis=(0, 2, 3), dtype=np.float64).astype(np.float32)  # (C,)
    nmu_arr = np.ascontiguousarray((-ymu).reshape(2, 128).T)  # (128, 2)
    in_maps = []
    for n in range(N):
        in_maps.append({
            "xs": np.ascontiguousarray(x[n].reshape(C, HW)),
            "ys": np.ascontiguousarray(y[n].reshape(C, HW)),
            "nmu": nmu_arr,
        })
    nc = _get_nc()
    res = run_bass_kernel_spmd(nc, in_maps, core_ids=list(range(N)), trace=trace)
    ccx = np.array([res.results[n]["out"][0, 0] for n in range(N)], dtype=np.float32)
    return ccx, res


def kernel(x, y):
    ccx, _ = run_device(x, y)
    loss = float(np.mean(-np.log(ccx.astype(np.float64) + EPS)))
    return np.float32(loss)


if __name__ == "__main__":
    rng = np.random.default_rng(0)
    x = rng.standard_normal((N, C, H, W), dtype=np.float32)
    y = rng.standard_normal((N, C, H, W), dtype=np.float32)
    print("loss:", kernel(x, y))


# revision 14
# speedup vs baseline: 1.7484x; 1.0002x over previous
"""CCX loss kernel for Trainium2 (8 NeuronCores, data-parallel over batch).

Math (per batch element n, C=256 channels, HW=64*64=4096 pixels):
  y_mu[c]   = mean over (n, h, w) of y            (host, tiny)
  x_c = x - y_mu ; y_c = y - y_mu
  x_n = x_c/||x_c||_C ; y_n = y_c/||y_c||_C
  s[i,j]    = sum_c x_n[c,i] y_n[c,j]
  d = 1-s ; dt = d/(dmin_i+eps) ; w = exp((1-dt)/0.5)
  ccx_ij = w/sum_j w ; ccx_n = mean_j max_i ccx_ij
  loss = mean_n -log(ccx_n + eps)                 (host, 8 scalars)

Device identities:
  u_ij = exp(alpha_i*G_ij) / Z_i   with  G = xc^T yn (unnormalized-x),
     alpha_i = 2*invx_i/(dmin_i+eps),  Z_i = sum_j exp(alpha_i G_ij)
  ccx_n = mean_j max_i u_ij

Schedule (per 128-row block r of G):
  A-stream: fp8e4 DoubleRow matmuls (full K=256 in one matmul) compute
    G quarters in PSUM (2 banks, double buffered) -> DVE rowmax -> free.
    Stats chain gives alpha_r.
  B-stream: regenerate the same G quarters (PE is cheap: 256 cyc per
    512-col DR matmul) -> Act exp(scale=alpha_r) writes E_r to SBUF
    bf16 with accum -> Z_r.
  ACC update (one fused DVE op, all-SBUF bf16):
    ACC = max(ACC, E_r * (1/Z_r))      [scalar_tensor_tensor]
  Finally 32 PE transposes of ACC give the cross-partition (over i)
  column max via cheap [128,128] reduces; ccx = mean_j ACC-max.

Accuracy: fp8e4m3 inputs to the similarity matmul, bf16 E/U. CPU
simulation vs the fp32 reference gives rel err ~2e-3 (gate is 2e-2).
"""

import os
import sys

import numpy as np

sys.path.insert(0, "/opt/trn_rl_repo")
os.environ.setdefault("JAX_PLATFORMS", "axon")

import concourse.mybir as mybir
import concourse.tile as tile
from concourse import bacc, bass_isa
from concourse import bass_utils as _bu
from concourse.bass_utils import run_bass_kernel_spmd
from concourse.masks import make_identity



N, C, H, W = 8, 256, 64, 64
HW = H * W          # 4096
NB = HW // 128      # 32 blocks of 128 rows
EPS = 1e-6
F32 = mybir.dt.float32
BF16 = mybir.dt.bfloat16
FP8 = mybir.dt.float8e4
ALU = mybir.AluOpType
ACTF = mybir.ActivationFunctionType
AX = mybir.AxisListType
DR = mybir.MatmulPerfMode.DoubleRow

_cached = {}


def _build():
    nc = bacc.Bacc(None, target_bir_lowering=False, debug=True)
    xs = nc.dram_tensor("xs", [C, HW], F32, kind="ExternalInput")
    ys = nc.dram_tensor("ys", [C, HW], F32, kind="ExternalInput")
    nmu = nc.dram_tensor("nmu", [128, 2], F32, kind="ExternalInput")  # -mean
    out = nc.dram_tensor("out", [1, 1], F32, kind="ExternalOutput")
    scr_y = nc.dram_tensor("scr_y", [NB, 128], F32)
    scr_n = nc.dram_tensor("scr_n", [2, HW], F32)

    import concourse.bass as bass_mod

    with tile.TileContext(nc) as tc:
        with (
            tc.tile_pool(name="big", bufs=1) as big,
            tc.tile_pool(name="bc", bufs=1) as bc,
            tc.tile_pool(name="sq", bufs=2) as sqp,
            tc.tile_pool(name="eb", bufs=2) as ebp,
            tc.tile_pool(name="sm", bufs=1) as sm,
            tc.tile_pool(name="mmq", bufs=1, space="PSUM") as mmq,
        ):
            # ---------------- load ----------------
            x = big.tile([128, 2, HW], F32, tag="x")
            y = big.tile([128, 2, HW], F32, tag="y")
            xc8 = big.tile([128, 2, HW], FP8, tag="xc8")
            yn8 = big.tile([128, 2, HW], FP8, tag="yn8")
            acc = big.tile([128, HW], BF16, tag="acc")
            nmu_sb = sm.tile([128, 2], F32, tag="nmu")
            nc.sync.dma_start(out=y[:, :, :], in_=ys.rearrange("(g p) j -> p g j", p=128))
            nc.sync.dma_start(out=nmu_sb[:, :], in_=nmu[:, :])
            nc.sync.dma_start(out=x[:, :, :], in_=xs.rearrange("(g p) j -> p g j", p=128))

            ones_col = sm.tile([128, 1], BF16, tag="ones_col")
            nc.vector.memset(ones_col[:, :], 1.0)
            nc.vector.memset(acc[:, :], 0.0)

            # ---------------- channel sumsq -> 1/norm --------------------
            # sq = (t - mu)^2 via Act Square (bf16) with per-partition
            # bias; ones-STATIONARY matmuls (trivial weight loads)
            # contract the channel groups into PSUM row slices [1, 512],
            # which bounce through DRAM into column layout.  y first
            # (invy gates yn8 which gates everything).
            for ti, src in ((1, y), (0, x)):
                rowbuf = sm.tile([1, HW], F32, tag=f"rowbuf{ti}")
                for ch in range(4):
                    sq = sqp.tile([128, 2, 1024], BF16, tag="sqt")
                    for g in range(2):
                        nc.scalar.activation(
                            out=sq[:, g, :],
                            in_=src[:, g, 1024 * ch : 1024 * (ch + 1)],
                            func=ACTF.Square, bias=nmu_sb[:, g : g + 1])
                    pt = mmq.tile([128, 2, 512], F32, tag=f"qa{ch % 2}",
                                  name=f"nsq_{ti}_{ch}")
                    for cc in range(2):
                        for g in range(2):
                            nc.tensor.matmul(
                                pt[0:1, cc, :],
                                ones_col[:, :],
                                sq[:, g, 512 * cc : 512 * (cc + 1)],
                                start=(g == 0), stop=(g == 1))
                    nc.scalar.copy(
                        rowbuf[0:1, 1024 * ch : 1024 * (ch + 1)],
                        pt[0:1, :, :])
                nc.sync.dma_start(out=scr_n[ti : ti + 1, :], in_=rowbuf[:, :])
            nsq = sm.tile([128, 64], F32, tag="nsq")
            norms = sm.tile([128, 64], F32, tag="norms")
            invc = sm.tile([128, 64], F32, tag="invc")
            nc.sync.dma_start(
                out=nsq[:, 32:64],
                in_=scr_n[1, :].rearrange("(r p) -> p r", p=128))
            nc.scalar.activation(
                out=norms[:, 32:64], in_=nsq[:, 32:64], func=ACTF.Sqrt)
            nc.vector.reciprocal(invc[:, 32:64], norms[:, 32:64])
            nc.sync.dma_start(
                out=scr_y[:, :].rearrange("r p -> p r"), in_=invc[:, 32:64])
            nc.sync.dma_start(
                out=nsq[:, 0:32],
                in_=scr_n[0, :].rearrange("(r p) -> p r", p=128))
            nc.scalar.activation(
                out=norms[:, 0:32], in_=nsq[:, 0:32], func=ACTF.Sqrt)
            nc.vector.reciprocal(invc[:, 0:32], norms[:, 0:32])
            neginvx = sm.tile([128, 32], F32, tag="neginvx")
            nc.vector.tensor_scalar(
                out=neginvx[:, :], in0=invc[:, 0:32], scalar1=-1.0,
                scalar2=None, op0=ALU.mult)
            twoinvx = sm.tile([128, 32], F32, tag="twoinvx")
            nc.vector.tensor_scalar(
                out=twoinvx[:, :], in0=invc[:, 0:32], scalar1=2.0,
                scalar2=None, op0=ALU.mult)

            # invy broadcast along partitions (DRAM bounce)
            invybc = bc.tile([128, HW], F32, tag="invybc")
            for cc in range(4):
                bcast_src_y = bass_mod.AP(
                    tensor=scr_y[:, :].tensor, offset=1024 * cc,
                    ap=[[0, 128], [1, 1024]])
                nc.sync.dma_start(
                    out=invybc[:, 1024 * cc : 1024 * (cc + 1)], in_=bcast_src_y)

            # ---------------- fp8 operands ----------------
            # yn8 = (y - mu) * invy  (fused STT, chunked so the first row
            # block's matmuls can start before the full conversion lands)
            for cc in range(4):
                sl = slice(1024 * cc, 1024 * (cc + 1))
                for g in range(2):
                    nc.vector.scalar_tensor_tensor(
                        out=yn8[:, g, sl], in0=y[:, g, sl],
                        scalar=nmu_sb[:, g : g + 1], in1=invybc[:, sl],
                        op0=ALU.add, op1=ALU.mult)
            # xc8 = x - mu (Act: per-partition bias, fp8 out; keeps DVE free)
            for g in range(2):
                nc.scalar.activation(
                    out=xc8[:, g, :], in_=x[:, g, :],
                    func=ACTF.Identity, bias=nmu_sb[:, g : g + 1])

            # ---------------- main loop over row blocks -------------------
            gacc = sm.tile([128, 128], F32, tag="gacc")
            zacc = sm.tile([128, 128], F32, tag="zacc")
            gmaxc = sm.tile([128, 32], F32, tag="gmaxc")
            tmpc = sm.tile([128, 32], F32, tag="tmpc")
            reccol = sm.tile([128, 32], F32, tag="reccol")
            acol = sm.tile([128, 32], F32, tag="acol")
            zsum = sm.tile([128, 32], F32, tag="zsum")
            zinv = sm.tile([128, 32], F32, tag="zinv")
            ebufs = {}

            def tail_ops(r):
                # Z_r, 1/Z_r, ACC = max(ACC, E_r/Z_r); emitted one row late
                # so the in-order DVE queue never waits on Act.
                nc.vector.reduce_sum(
                    zsum[:, r : r + 1],
                    zacc[:, 4 * r : 4 * r + 4], axis=AX.X)
                nc.vector.reciprocal(zinv[:, r : r + 1], zsum[:, r : r + 1])
                nc.vector.scalar_tensor_tensor(
                    out=acc[:, :], in0=ebufs[r % 2][:, :],
                    scalar=zinv[:, r : r + 1], in1=acc[:, :],
                    op0=ALU.mult, op1=ALU.max)

            # dummy bf16 weights: standalone LDWEIGHTS issued while the PE
            # waits on PSUM frees keep it continuously busy, so the clock
            # ramps to (and stays at) the 2.4 GHz pstate.
            wdum = sm.tile([128, 128], BF16, tag="wdum")
            nc.vector.memset(wdum[:, :], 0.0)

            for r in range(NB):
                lhs = xc8[:, :, 128 * r : 128 * (r + 1)]
                # ---- A-stream: G quarters + rowmax ----
                for q in range(4):
                    qa = mmq.tile([128, 2, 512], F32, tag=f"qa{q % 2}",
                                  name=f"qa_{r}_{q}")
                    for cc in range(2):
                        c = 2 * q + cc
                        nc.tensor.matmul(
                            qa[:, cc, :], lhs,
                            yn8[:, :, 512 * c : 512 * (c + 1)],
                            start=True, stop=True, perf_mode=DR)
                    nc.vector.reduce_max(
                        gacc[:, 4 * r + q : 4 * r + q + 1],
                        qa[:, :, :], axis=AX.XY)
                # keep the PE busy through the stats gate (pstate warm)
                for _ in range(10):
                    nc.tensor.ldweights(wdum[:, :])
                # ---- stats chain -> alpha_r ----
                nc.vector.reduce_max(
                    gmaxc[:, r : r + 1], gacc[:, 4 * r : 4 * r + 4], axis=AX.X)
                nc.vector.tensor_scalar(
                    out=tmpc[:, r : r + 1], in0=gmaxc[:, r : r + 1],
                    scalar1=neginvx[:, r : r + 1], scalar2=float(1.0 + EPS),
                    op0=ALU.mult, op1=ALU.add)
                nc.vector.reciprocal(reccol[:, r : r + 1], tmpc[:, r : r + 1])
                nc.vector.tensor_scalar(
                    out=acol[:, r : r + 1], in0=reccol[:, r : r + 1],
                    scalar1=twoinvx[:, r : r + 1], scalar2=None, op0=ALU.mult)
                # ---- B-stream: regen + exp -> E_r (bf16), Z accum ----
                eb = ebp.tile([128, HW], BF16, tag=f"eb{r % 2}",
                              name=f"eb_{r}")
                ebufs[r % 2] = eb
                for h in range(4):
                    qb = mmq.tile([128, 2, 512], F32, tag=f"qb{h % 2}",
                                  name=f"qb_{r}_{h}")
                    for cc in range(2):
                        c = 2 * h + cc
                        nc.tensor.matmul(
                            qb[:, cc, :], lhs,
                            yn8[:, :, 512 * c : 512 * (c + 1)],
                            start=True, stop=True, perf_mode=DR)
                    nc.scalar.activation(
                        out=eb[:, 1024 * h : 1024 * (h + 1)], in_=qb[:, :, :],
                        func=ACTF.Exp, scale=acol[:, r : r + 1],
                        accum_out=zacc[:, 4 * r + h : 4 * r + h + 1])
                if r > 0:
                    tail_ops(r - 1)
            tail_ops(NB - 1)

            # ---------------- final: cross-partition max of ACC -----------
            ident = sm.tile([128, 128], BF16, tag="ident")
            make_identity(nc, ident)
            mcol = sm.tile([128, 32], F32, tag="mcol")
            for jb in range(NB):
                tag = ("qa0", "qa1", "qb0", "qb1")[jb % 4]
                tp = mmq.tile([128, 2, 512], F32, tag=tag, name=f"tp_{jb}")
                tpb = tp[:, 0, 0:64].bitcast(BF16)
                nc.tensor.transpose(
                    tpb[:, 0:128], acc[:, 128 * jb : 128 * (jb + 1)],
                    ident[:, :])
                nc.vector.reduce_max(
                    mcol[:, jb : jb + 1], tpb[:, 0:128], axis=AX.X)
            msum = sm.tile([128, 1], F32, tag="msum")
            nc.vector.reduce_sum(msum[:, :], mcol[:, :], axis=AX.X)
            tot = sm.tile([128, 1], F32, tag="tot")
            nc.gpsimd.partition_all_reduce(
                tot[:, :], msum[:, :], channels=128,
                reduce_op=bass_isa.ReduceOp.add)
            res = sm.tile([1, 1], F32, tag="res")
            nc.vector.tensor_scalar(
                out=res[:, :], in0=tot[0:1, :], scalar1=float(1.0 / HW),
                scalar2=None, op0=ALU.mult)
            nc.sync.dma_start(out=out[:, :], in_=res[:, :])
    nc.compile()
    return nc


def _get_nc():
    if "nc" not in _cached:
        _cached["nc"] = _build()
    return _cached["nc"]


def run_device(x, y, trace=False):
    """x, y: (N, C, H, W) float32. Returns (ccx (N,), BassKernelResults)."""
    x = np.ascontiguousarray(np.asarray(x, dtype=np.float32))
    y = np.ascontiguousarray(np.asarray(y, dtype=np.float32))
    ymu = y.mean(axis=(0, 2, 3), dtype=np.float64).astype(np.float32)  # (C,)
    nmu_arr = np.ascontiguousarray((-ymu).reshape(2, 128).T)  # (128, 2)
    in_maps = []
    for n in range(N):
        in_maps.append({
            "xs": np.ascontiguousarray(x[n].reshape(C, HW)),
            "ys": np.ascontiguousarray(y[n].reshape(C, HW)),
            "nmu": nmu_arr,
        })
    nc = _get_nc()
    res = run_bass_kernel_spmd(nc, in_maps, core_ids=list(range(N)), trace=trace)
    ccx = np.array([res.results[n]["out"][0, 0] for n in range(N)], dtype=np.float32)
    return ccx, res


def kernel(x, y):
    ccx, _ = run_device(x, y)
    loss = float(np.mean(-np.log(ccx.astype(np.float64) + EPS)))
    return np.float32(loss)


if __name__ == "__main__":
    rng = np.random.default_rng(0)
    x = rng.standard_normal((N, C, H, W), dtype=np.float32)
    y = rng.standard_normal((N, C, H, W), dtype=np.float32)
    print("loss:", kernel(x, y))
